# revision 35
# baseline (speedup 1.0000x reference)
"""Trainium2 Bass kernel for causal multi-head attention with RoPE.

Sharding: tensor-parallel over heads. 16 heads / 8 cores = 2 heads per core.
Each core computes QKV projection for its 2 heads (full sequence), RoPE,
causal flash-style attention, and the output projection rows belonging to
its heads (the reference's permute+reshape makes per-head output row slices
disjoint, so no cross-core reduction is needed).

All matmuls run in bf16 (1 cycle/row on the PE, same rate as f32r, half the
DMA/SBUF footprint) accumulating in fp32 PSUM.  Scores are computed
transposed (keys on partitions) so attn@V needs no transposes.

Elementwise work (ACT exp / DVE / GpSimd) is the second wall after the PE:
every per-instruction overhead is amortized by operating on PAIRS of PSUM
banks ([128, 2, 512] tiles), and the softmax denominator is split between
the PE (ones-matmul over odd key-blocks) and the DVE (accumulate over even
key-blocks, folded in with one extra matmul).

Schedule: warmup matmuls (HAM un-throttle) -> b0 proj -> b0 attn -> b1 proj
-> b1 attn -> joint output projection for both batches (out_w streamed once,
prefetched during attention).
"""

import math
import os
import sys

for _p in ("/opt/trn_rl_repo",):
    if _p not in sys.path and os.path.isdir(_p):
        sys.path.insert(0, _p)

import numpy as np
import ml_dtypes

import concourse.bass as bass  # noqa: F401  (AP helpers)
import concourse.mybir as mybir
import concourse.tile as tile
from concourse import bacc
from concourse.bass_utils import run_bass_kernel_spmd

F32 = mybir.dt.float32
F32R = mybir.dt.float32r
BF16 = mybir.dt.bfloat16
NPBF16 = ml_dtypes.bfloat16

B, T, C = 2, 2048, 2048
H, D = 16, 128
N_CORES = 8
HPC = H // N_CORES          # heads per core (2)
BT = B * T                  # 4096
KC = C // 128               # 16 contraction blocks
TB = 512                    # token block (proj + attention)
NTB = T // TB               # 4 t-blocks per batch
SCALE = 1.0 / math.sqrt(D)
OJ = 512                    # out-proj column block

_CACHED_NC = None


def build_nc():
    nc = bacc.Bacc("TRN2", target_bir_lowering=False)

    xT = nc.dram_tensor("xT", [C, BT], BF16, kind="ExternalInput")
    wqkT = nc.dram_tensor("wqkT", [C, 4 * 128], BF16, kind="ExternalInput")
    wvT = nc.dram_tensor("wvT", [C, 2 * 128], BF16, kind="ExternalInput")
    owF = nc.dram_tensor("owF", [C, C], BF16, kind="ExternalInput")
    cosF = nc.dram_tensor("cosF", [128, 2, T], BF16, kind="ExternalInput")
    sinS = nc.dram_tensor("sinS", [128, 2, T], BF16, kind="ExternalInput")
    onesI = nc.dram_tensor("onesI", [128, 128], F32R, kind="ExternalInput")
    y = nc.dram_tensor("y", [B * HPC, 128, C], F32, kind="ExternalOutput")

    with tile.TileContext(nc) as tc:
        with tc.tile_pool(name="wpool", bufs=1) as wpool, \
             tc.tile_pool(name="xpool", bufs=16) as xpool, \
             tc.tile_pool(name="rotpool", bufs=1) as rotpool, \
             tc.tile_pool(name="vpool", bufs=1) as vpool, \
             tc.tile_pool(name="apool", bufs=1) as apool, \
             tc.tile_pool(name="epool", bufs=3) as epool, \
             tc.tile_pool(name="dpool", bufs=2) as dpool, \
             tc.tile_pool(name="tpool", bufs=2) as tpool, \
             tc.tile_pool(name="rpool", bufs=2) as rpool, \
             tc.tile_pool(name="ypool", bufs=2) as ypool, \
             tc.tile_pool(name="owpool", bufs=1) as owpool, \
             tc.tile_pool(name="bigps", bufs=2, space="PSUM") as bigps, \
             tc.tile_pool(name="attps", bufs=2, space="PSUM") as attps, \
             tc.tile_pool(name="denps", bufs=2, space="PSUM") as denps:

            # per-k weight tiles: dependency tracking is whole-tile, so the
            # k-th matmul group must only wait for its own 128KB chunk
            twqk = [wpool.tile([128, 4 * 128], BF16, tag=f"twqk{k}",
                               name=f"twqk{k}") for k in range(KC)]
            twv = [wpool.tile([128, 8, 2 * 128], BF16, tag=f"twv{g}",
                              name=f"twv{g}") for g in range(2)]
            tcf = wpool.tile([128, 2, T], BF16)
            tsn = wpool.tile([128, 2, T], BF16)
            tones = wpool.tile([128, 128], F32R)
            tonesb = wpool.tile([128, 128], BF16)
            scr = wpool.tile([128, 512], BF16)
            wqkr = wqkT.rearrange("(kb p) m -> p kb m", p=128)
            wvr = wvT.rearrange("(kb p) m -> p kb m", p=128)

            # PE warm-up: matmuls on a zeroed scratch tile keep the PE busy
            # while the first x/weight DMAs land, lifting HAM to 8/8 before
            # the real work starts (HAM window ~3.4us; first data ~10-12us).
            nc.vector.memset(scr[:], 0.0)
            nc.vector.memset(tonesb[:], 1.0)
            for _ in range(16):
                psw = bigps.tile([128, 2, TB], F32, tag="flow")
                nc.tensor.matmul(psw[:, 0, :], scr[:, 0:128], scr[:],
                                 start=True, stop=True)

            for k in range(2):
                nc.sync.dma_start(twqk[k][:], wqkr[:, k, :])

            atn = [apool.tile([128, T], BF16, tag=f"attnT{i}",
                              name=f"attnT{i}") for i in range(B * HPC)]
            owFr = owF[:, :].rearrange("(u p) j -> p u j", p=128)
            owjs = [owpool.tile([128, KC, OJ], BF16, tag=f"owj{jb}",
                                name=f"owj{jb}") for jb in range(C // OJ)]

            # Output-projection groups are interleaved, a few matmuls at a
            # time, into the PE slack of the ACT-bound attention units.
            # reference applies permute(0,2,1,3).reshape(B,T,C) to a
            # [B,T,H,D] tensor: out row t' = h*128 + t//16 uses head h,
            # col c' = (t%16)*128 + d.  Y_slice = attn_h.reshape(128,
            # 16*128) @ out_w.T, contracting over (u=t%16, d).
            oj_queue = []           # flat (i, jb, u) work items
            oj_state = {"psy": None}

            def queue_oj_groups(i):
                for jb in range(C // OJ):
                    for u in range(KC):
                        oj_queue.append((i, jb, u))

            def emit_oj(n):
                for _ in range(min(n, len(oj_queue))):
                    i, jb, u = oj_queue.pop(0)
                    if u == 0:
                        oj_state["psy"] = denps.tile([128, TB], F32,
                                                     tag="psden",
                                                     name="psy")
                    psy = oj_state["psy"]
                    av = atn[i][:].rearrange("p (a u) -> p a u", u=16)
                    nc.tensor.matmul(
                        psy[:], av[:, :, u], owjs[jb][:, u, :],
                        start=(u == 0), stop=(u == KC - 1))
                    if u == KC - 1:
                        ys = ypool.tile([128, OJ], F32, tag="ys")
                        nc.vector.tensor_copy(ys[:], psy[:])
                        nc.sync.dma_start(
                            y[i, :, jb * OJ:(jb + 1) * OJ], ys[:])

            def emit_attn_unit(b, h, tb, rots2, vts, slots):
                ts_sl = slice(tb * TB, (tb + 1) * TB)
                ns = (tb + 1) * (TB // 128)
                ps_att = attps.tile([128, TB], F32, tag="psatt")
                ps_den = denps.tile([128, TB], F32, tag="psden")
                den = dpool.tile([128, TB], F32R, tag="den")
                state = {"first_odd": True}
                pend = []

                def flush():
                    et2, s0, r0, s1, r1 = pend.pop(0)
                    nc.tensor.matmul(
                        ps_att[:, r0:],
                        vts[s0 // 4][:, s0 % 4, h * 128:(h + 1) * 128],
                        et2[:, 0, r0:], start=(s0 == 0), stop=False)
                    nc.tensor.matmul(
                        ps_att[:, r1:],
                        vts[s1 // 4][:, s1 % 4, h * 128:(h + 1) * 128],
                        et2[:, 1, r1:], start=False, stop=(s1 == ns - 1))
                    # odd key-blocks' denominator contribution on PE
                    nc.tensor.matmul(
                        ps_den[:, r1:], tonesb[:], et2[:, 1, r1:],
                        start=state["first_odd"], stop=False)
                    state["first_odd"] = False

                for pi in range(ns // 2):
                    si0, si1 = 2 * pi, 2 * pi + 1
                    diag0 = si0 >= ns - TB // 128
                    diag1 = si1 >= ns - TB // 128
                    r0 = si0 * 128 - tb * TB if diag0 else 0
                    r1 = si1 * 128 - tb * TB if diag1 else 0
                    ps2 = bigps.tile([128, 2, TB], F32, tag="flow")
                    for j, si, r in ((0, si0, r0), (1, si1, r1)):
                        o = (si % 4) * 128
                        nc.tensor.matmul(
                            ps2[:, j, r:],
                            rots2[1][si // 4][:, h, o:o + 128],
                            rots2[0][tb][:, h, r:],
                            start=True, stop=True)
                    et2 = epool.tile([128, 2, TB], BF16, tag="et")
                    # one exp over both banks, starting at the first valid
                    # column; cols left of the causal line are garbage but
                    # never read
                    ef = et2[:].rearrange("p a q -> p (a q)")
                    sf = ps2[:].rearrange("p a q -> p (a q)")
                    nc.scalar.activation(
                        ef[:, r0:], sf[:, r0:],
                        mybir.ActivationFunctionType.Exp, scale=SCALE)
                    for j, si, r, dg in ((0, si0, r0, diag0),
                                         (1, si1, r1, diag1)):
                        if dg:
                            nc.gpsimd.affine_select(
                                out=et2[:, j, r:r + 128],
                                in_=et2[:, j, r:r + 128],
                                compare_op=mybir.AluOpType.is_ge,
                                fill=0.0, base=0, pattern=[[1, 128]],
                                channel_multiplier=-1)
                    # even key-blocks' denominator on DVE
                    if si0 == 0:
                        nc.vector.tensor_copy(den[:], et2[:, 0, :])
                    else:
                        nc.vector.tensor_add(
                            out=den[:, r0:], in0=den[:, r0:],
                            in1=et2[:, 0, r0:])
                    pend.append((et2, si0, r0, si1, r1))
                    if len(pend) > 1:
                        flush()
                    # spread output-projection matmuls over the standalone
                    # phase's remaining score-pair slots
                    if slots is not None and oj_queue:
                        emit_oj(-(-len(oj_queue) // slots[0]))
                        slots[0] -= 1
                while pend:
                    flush()
                # fold the DVE half in and broadcast the total
                nc.tensor.matmul(ps_den[:], tones[:], den[:],
                                 start=False, stop=True)
                rcp = rpool.tile([128, TB], F32, tag="rcp")
                nc.vector.reciprocal_approx_fast(out=rcp[:], in_=ps_den[:])
                nc.vector.tensor_mul(
                    out=atn[b * HPC + h][:, ts_sl],
                    in0=ps_att[:], in1=rcp[:])

            for b in range(B):
                # ---------------- QKV projection + RoPE ----------------
                # h=0 attention units are woven between projection token
                # blocks: the projection keeps the PE dense while the units'
                # exp/DVE tails hide under it.
                # rots2[0][tb] = q pair (h0,h1); rots2[1][tb] = k pair
                rots2 = [[rotpool.tile([128, 2, TB], BF16, tag=f"rot{m}_{j}",
                                       name=f"rot{m}_{j}")
                          for j in range(NTB)]
                         for m in range(2)]
                vts = [vpool.tile([128, 4, 2 * 128], BF16, tag=f"vt{j}",
                                  name=f"vt{j}") for j in range(NTB)]
                for tb in range(NTB):
                    c0 = b * T + tb * TB
                    ts_sl = slice(tb * TB, (tb + 1) * TB)
                    xTr = xT[:, c0:c0 + TB].rearrange(
                        "(kb p) t -> p kb t", p=128)
                    xq = []
                    for g in range(KC // 2):
                        xg = xpool.tile([128, 2, TB], BF16, tag="xk",
                                        name="xg")
                        nc.sync.dma_start(xg[:], xTr[:, 2 * g:2 * g + 2, :])
                        xq.append(xg)
                    xk = [xq[k // 2][:, k % 2, :] for k in range(KC)]
                    if b == 0 and tb == 0:
                        # deferred loads, behind the first x block; per-k
                        # tiles let tb0's k-loop consume chunks as they land
                        for k in range(2, KC):
                            nc.sync.dma_start(twqk[k][:], wqkr[:, k, :])
                        nc.sync.dma_start(tcf[:], cosF[:, :, :])
                        nc.sync.dma_start(tsn[:], sinS[:, :, :])
                        for g in range(2):
                            nc.sync.dma_start(
                                twv[g][:], wvr[:, g * 8:(g + 1) * 8, :])
                        nc.sync.dma_start(tones[:], onesI[:, :])
                    for mp in range(2):           # m-pairs: (q0,q1), (k0,k1)
                        ps2 = bigps.tile([128, 2, TB], F32, tag="flow")
                        for j in range(2):
                            m = 2 * mp + j
                            for k in range(KC):
                                nc.tensor.matmul(
                                    ps2[:, j, :],
                                    twqk[k][:, m * 128:(m + 1) * 128],
                                    xk[k], start=(k == 0), stop=(k == KC - 1))
                        # RoPE on the [128, 2, 512] pair.  rows 0:64 = x1,
                        # 64:128 = x2 of each head tensor; the half-swap is
                        # done by two ACT copies straight out of PSUM.
                        qsw = tpool.tile([128, 2, TB], BF16, tag="qsw")
                        nc.scalar.copy(qsw[0:64, :, :], ps2[64:128, :, :])
                        nc.scalar.copy(qsw[64:128, :, :], ps2[0:64, :, :])
                        pc = tpool.tile([128, 2, TB], BF16, tag="pc")
                        nc.vector.tensor_mul(out=pc[:], in0=ps2[:, :, :],
                                             in1=tcf[:, :, ts_sl])
                        pn = tpool.tile([128, 2, TB], BF16, tag="pn")
                        nc.gpsimd.tensor_mul(out=pn[:], in0=qsw[:],
                                             in1=tsn[:, :, ts_sl])
                        nc.vector.tensor_add(
                            out=rots2[mp][tb][:], in0=pc[:], in1=pn[:])
                    for tsp in range(2):          # ts-pairs: (0,1), (2,3)
                        psv = bigps.tile([128, 2, TB], F32, tag="flow")
                        for j in range(2):
                            ts = 2 * tsp + j
                            for k in range(KC):
                                nc.tensor.matmul(
                                    psv[:, j, 0:256],
                                    xk[k][:, ts * 128:(ts + 1) * 128],
                                    twv[k // 8][:, k % 8, :],
                                    start=(k == 0), stop=(k == KC - 1))
                        nc.vector.tensor_copy(
                            vts[tb][:, 2 * tsp:2 * tsp + 2, :],
                            psv[:, :, 0:256])
                    if tb >= 2:
                        emit_attn_unit(b, 0, tb - 2, rots2, vts, None)

                if b == 0:
                    # stream out_w into SBUF (resident for the injected
                    # output-projection groups)
                    for jb in range(C // OJ):
                        nc.sync.dma_start(
                            owjs[jb][:], owFr[:, :, jb * OJ:(jb + 1) * OJ])
                emit_attn_unit(b, 0, 2, rots2, vts, None)
                emit_attn_unit(b, 0, 3, rots2, vts, None)

                # ---------------- h=1 attention phase ----------------
                # atn[2b] finished above; inject its out-proj groups here
                queue_oj_groups(2 * b)
                slots = [20]
                for tb in range(NTB):
                    emit_attn_unit(b, 1, tb, rots2, vts, slots)
                emit_oj(len(oj_queue))   # drain phase leftovers

            # -------- output projection: remaining groups --------
            queue_oj_groups(1)
            queue_oj_groups(3)
            emit_oj(len(oj_queue))
    nc.compile()
    return nc


def _get_nc():
    global _CACHED_NC
    if _CACHED_NC is None:
        _CACHED_NC = build_nc()
    return _CACHED_NC


def _rope_tables():
    pos = np.arange(T, dtype=np.float64)[:, None]
    div = np.exp(np.arange(0, D, 2, dtype=np.float64) *
                 (-math.log(10000.0) / D))
    ang = pos * div  # [T, 64]
    sinT = np.sin(ang).T.astype(np.float32)  # [64, T]
    cosT = np.cos(ang).T.astype(np.float32)
    cosF = np.concatenate([cosT, cosT], axis=0)          # [128, T]
    sinS = np.concatenate([-sinT, sinT], axis=0)         # [128, T]
    # duplicated along a pair dim so [128, 2, 512] slices broadcast over
    # head pairs
    cos2 = np.ascontiguousarray(
        np.broadcast_to(cosF[:, None, :], (128, 2, T))).astype(NPBF16)
    sin2 = np.ascontiguousarray(
        np.broadcast_to(sinS[:, None, :], (128, 2, T))).astype(NPBF16)
    return cos2, sin2


def make_in_maps(x, qkv_w, out_w):
    xT = np.ascontiguousarray(x.reshape(BT, C).T).astype(NPBF16)
    owF = np.ascontiguousarray(out_w.T).astype(NPBF16)
    cosF, sinS = _rope_tables()
    ones = np.ones((128, 128), dtype=np.float32)
    in_maps = []
    for c in range(N_CORES):
        h0, h1 = 2 * c, 2 * c + 1
        wqk = np.concatenate([
            qkv_w[h0 * D:(h0 + 1) * D],
            qkv_w[h1 * D:(h1 + 1) * D],
            qkv_w[C + h0 * D:C + (h0 + 1) * D],
            qkv_w[C + h1 * D:C + (h1 + 1) * D],
        ], axis=0)                       # [512, 2048]
        wv = np.concatenate([
            qkv_w[2 * C + h0 * D:2 * C + (h0 + 1) * D],
            qkv_w[2 * C + h1 * D:2 * C + (h1 + 1) * D],
        ], axis=0)                       # [256, 2048]
        in_maps.append({
            "xT": xT,
            "wqkT": np.ascontiguousarray(wqk.T).astype(NPBF16),
            "wvT": np.ascontiguousarray(wv.T).astype(NPBF16),
            "owF": owF,
            "cosF": cosF,
            "sinS": sinS,
            "onesI": ones,
        })
    return in_maps


def kernel(x, qkv_w, out_w, _trace=False, _trace_kwargs=None):
    x = np.asarray(x, dtype=np.float32)
    qkv_w = np.asarray(qkv_w, dtype=np.float32)
    out_w = np.asarray(out_w, dtype=np.float32)
    nc = _get_nc()
    in_maps = make_in_maps(x, qkv_w, out_w)
    kwargs = {}
    if _trace:
        kwargs["trace"] = True
        if _trace_kwargs:
            kwargs.update(_trace_kwargs)
    res = run_bass_kernel_spmd(nc, in_maps, core_ids=list(range(N_CORES)),
                               **kwargs)
    out = np.empty((B, T, C), dtype=np.float32)
    for c in range(N_CORES):
        yc = res.results[c]["y"]  # [B*HPC, 128, C]
        for b in range(B):
            for hl in range(HPC):
                hg = HPC * c + hl
                out[b, hg * 128:(hg + 1) * 128] = yc[b * HPC + hl]
    if _trace:
        return out, res
    return out
